# revision 2
# baseline (speedup 1.0000x reference)
"""CTRNN Trainium2 kernel (8-core SPMD, batch-sharded data-parallel).

Math (per reference):
    xp_t   = x_t @ W_in.T + b_in                      # precomputed for all t
    pre_t  = h_t @ W_hh.T + b_hh + xp_t
    h_{t+1}= (1-a)*h_t + a*sigmoid(pre_t),  a = 0.2

Device-side substitution: g = h / a, W' = a * W_hh  =>
    pre_t  = g_t @ W'.T + (xp_t + b_hh + b_in)
    g_{t+1}= (1-a)*g_t + sigmoid(pre_t)
    h_t    = a * g_t   (scaled on host)

Layouts (per core, batch shard BC=8):
    xp   : SBUF [128, 8*T*BC]  bf16, col = m*T*BC + t*BC + b   (m = H/128 tile)
    W'^T : SBUF [128, 8*1024]  bf16, col = k*1024 + j          (k = H_in/128 chunk)
    g^T  : SBUF [128, 8*BC]    bf16 x2 (double buffer), col = k*BC + b
    recurrence:  psum[128, BC] (m) = sum_k W'^T[k,m-tile].T @ g^T[k]
"""

import numpy as np

T = 512
B = 64
I = 512
H = 1024
NCORES = 8
BC = B // NCORES          # batch per core
ALPHA = 20.0 / 100.0
OMA = 1.0 - ALPHA
GAIN = 1.0
KT = H // 128             # 8 k-chunks / m-tiles
KI = I // 128             # 4 input k-chunks
UNROLL = 16


def build_nc(Tn=T, unroll=UNROLL):
    import concourse.bacc as bacc
    import concourse.mybir as mybir
    import concourse.tile as tile
    from concourse.bass import ds
    from concourse.masks import make_identity

    FP32 = mybir.dt.float32
    BF16 = mybir.dt.bfloat16
    AF = mybir.ActivationFunctionType
    OP = mybir.AluOpType

    R = Tn * BC              # rows of flattened x
    assert R % 128 == 0
    RT = R // 128            # 128-row tiles of x
    assert Tn % unroll == 0
    niter = Tn // unroll

    nc = bacc.Bacc("TRN2", target_bir_lowering=False, debug=False,
                   num_devices=NCORES)
    x_d = nc.dram_tensor("x", (R, I), FP32, kind="ExternalInput")
    h0_d = nc.dram_tensor("hidden", (BC, H), FP32, kind="ExternalInput")
    win_d = nc.dram_tensor("w_in", (H, I), FP32, kind="ExternalInput")
    whh_d = nc.dram_tensor("w_hh", (H, H), FP32, kind="ExternalInput")
    bias_d = nc.dram_tensor("bias", (H,), FP32, kind="ExternalInput")
    out_d = nc.dram_tensor("outg", (H, R), FP32, kind="ExternalOutput")

    # persistent SBUF
    xp = nc.alloc_sbuf_tensor("xp_sb", [128, KT * R], BF16).ap()
    whT = nc.alloc_sbuf_tensor("whT_sb", [128, KT * H], BF16).ap()
    winT = nc.alloc_sbuf_tensor("winT_sb", [128, KI * H], FP32).ap()
    gbf = nc.alloc_sbuf_tensor("gbf_sb", [128, 2 * KT * BC], BF16).ap()
    gsb = nc.alloc_sbuf_tensor("gsb_sb", [128, KT * unroll * BC], FP32).ap()
    bias_sb = nc.alloc_sbuf_tensor("bias_sb", [128, KT], FP32).ap()

    with tile.TileContext(nc) as tc:
        with tc.tile_pool(name="consts", bufs=1) as consts:
            ident = consts.tile([128, 128], FP32)
            make_identity(nc, ident)

            # ---------------- phase 0: weights / bias / h0 ----------------
            with tc.tile_pool(name="wstage", bufs=2) as wstage, \
                 tc.tile_pool(name="tpsum", bufs=4, space="PSUM") as tpsum:
                # W_in^T (fp32): winT[p, ki*H + mt*128 + c] = W_in[mt*128+c, ki*128+p]
                for mt in range(KT):
                    ws = wstage.tile([128, I], FP32)
                    nc.sync.dma_start(out=ws[:, :], in_=win_d.ap()[mt * 128:(mt + 1) * 128, :])
                    for ki in range(KI):
                        pt = tpsum.tile([128, 128], FP32)
                        nc.tensor.transpose(pt[:, :], ws[:, ki * 128:(ki + 1) * 128], ident[:, :])
                        nc.scalar.activation(winT[:, ki * H + mt * 128: ki * H + mt * 128 + 128],
                                             pt[:, :], AF.Copy)
                # W'_hh^T (bf16, alpha folded): whT[p, k*H + jt*128 + c] = a*W_hh[jt*128+c, k*128+p]
                for jt in range(KT):
                    ws = wstage.tile([128, H], FP32)
                    nc.sync.dma_start(out=ws[:, :], in_=whh_d.ap()[jt * 128:(jt + 1) * 128, :])
                    for k in range(KT):
                        pt = tpsum.tile([128, 128], FP32)
                        nc.tensor.transpose(pt[:, :], ws[:, k * 128:(k + 1) * 128], ident[:, :])
                        nc.vector.tensor_scalar_mul(
                            whT[:, k * H + jt * 128: k * H + jt * 128 + 128],
                            pt[:, :], ALPHA)
                # bias = b_in + b_hh (host-combined), per-partition layout
                for mt in range(KT):
                    nc.sync.dma_start(
                        out=bias_sb[:, mt:mt + 1],
                        in_=bias_d.ap()[mt * 128:(mt + 1) * 128].rearrange("(p o) -> p o", o=1))
                # g0 = hidden^T / a  into slot (unroll-1); also bf16 buffer 0
                for k in range(KT):
                    hs = wstage.tile([128, BC], FP32)
                    nc.sync.dma_start(out=hs[:, :],
                                      in_=h0_d.ap()[:, k * 128:(k + 1) * 128].transpose([1, 0]))
                    slot = gsb[:, k * unroll * BC + (unroll - 1) * BC:
                               k * unroll * BC + unroll * BC]
                    nc.vector.tensor_scalar_mul(slot, hs[:, :], 1.0 / ALPHA)
                    nc.vector.tensor_copy(gbf[:, k * BC:(k + 1) * BC], slot)
                # preload sigmoid table before the hot loop
                warm = wstage.tile([128, KT], FP32)
                nc.scalar.activation(warm[:, :], bias_sb[:, :], AF.Sigmoid)

            # ---------------- phase 1: xp = x @ W_in.T + bias ----------------
            # groups of 4 row-tiles -> N=512 streams
            G = 4 if RT % 4 == 0 else (2 if RT % 2 == 0 else 1)
            NG = RT // G
            NCOL = G * 128
            with tc.tile_pool(name="xstage", bufs=3) as xstage, \
                 tc.tile_pool(name="xT", bufs=2) as xTpool, \
                 tc.tile_pool(name="tpsum2", bufs=4, space="PSUM") as tpsum2, \
                 tc.tile_pool(name="mpsum", bufs=2, space="PSUM") as mpsum:
                for grp in range(NG):
                    xT = xTpool.tile([128, KI * NCOL], FP32)
                    for rg in range(G):
                        rt = grp * G + rg
                        xs = xstage.tile([128, I], FP32)
                        nc.sync.dma_start(out=xs[:, :], in_=x_d.ap()[rt * 128:(rt + 1) * 128, :])
                        for ki in range(KI):
                            pt = tpsum2.tile([128, 128], FP32)
                            nc.tensor.transpose(pt[:, :], xs[:, ki * 128:(ki + 1) * 128], ident[:, :])
                            nc.scalar.activation(
                                xT[:, ki * NCOL + rg * 128: ki * NCOL + rg * 128 + 128],
                                pt[:, :], AF.Copy)
                    for mt in range(KT):
                        pm = mpsum.tile([128, NCOL], FP32)
                        for ki in range(KI):
                            nc.tensor.matmul(
                                pm[:, :],
                                winT[:, ki * H + mt * 128: ki * H + mt * 128 + 128],
                                xT[:, ki * NCOL:(ki + 1) * NCOL],
                                start=(ki == 0), stop=(ki == KI - 1))
                        nc.vector.tensor_scalar_add(
                            xp[:, mt * R + grp * NCOL: mt * R + (grp + 1) * NCOL],
                            pm[:, :], bias_sb[:, mt:mt + 1])

            # ---------------- phase 2: recurrence ----------------
            with tc.tile_pool(name="rpsum", bufs=KT, space="PSUM") as rpsum, \
                 tc.tile_pool(name="tails", bufs=4) as tails:
                with tc.For_i(0, niter, 1,
                              hint_engines=(mybir.EngineType.PE, mybir.EngineType.DVE)) as it:
                    base = it * (unroll * BC)
                    for j in range(unroll):
                        rbuf = (j % 2) * KT * BC          # read buffer offset
                        wbuf = ((j + 1) % 2) * KT * BC    # write buffer offset
                        jprev = (j - 1) % unroll
                        for mt in range(KT):
                            pm = rpsum.tile([128, BC], mybir.dt.float32)
                            for k in range(KT):
                                nc.tensor.matmul(
                                    pm[:, :],
                                    whT[:, k * H + mt * 128: k * H + mt * 128 + 128],
                                    gbf[:, rbuf + k * BC: rbuf + (k + 1) * BC],
                                    start=(k == 0), stop=(k == KT - 1))
                            pre = tails.tile([128, BC], mybir.dt.float32, tag="pre")
                            nc.vector.tensor_add(
                                pre[:, :], pm[:, :],
                                xp[:, ds(base + mt * R + j * BC, BC)])
                            s = tails.tile([128, BC], mybir.dt.float32, tag="sig")
                            nc.scalar.activation(s[:, :], pre[:, :], AF.Sigmoid)
                            gslot = gsb[:, mt * unroll * BC + j * BC:
                                        mt * unroll * BC + (j + 1) * BC]
                            gprev = gsb[:, mt * unroll * BC + jprev * BC:
                                        mt * unroll * BC + (jprev + 1) * BC]
                            nc.vector.scalar_tensor_tensor(
                                gslot, gprev, OMA, s[:, :], op0=OP.mult, op1=OP.add)
                            nc.vector.tensor_copy(
                                gbf[:, wbuf + mt * BC: wbuf + (mt + 1) * BC], gslot)
                    for mt in range(KT):
                        nc.sync.dma_start(
                            out=out_d.ap()[mt * 128:(mt + 1) * 128, ds(base, unroll * BC)],
                            in_=gsb[:, mt * unroll * BC:(mt + 1) * unroll * BC])

    nc.compile()
    return nc


# ---------------------------------------------------------------------------
# host side
# ---------------------------------------------------------------------------

_CACHE = {}


def _get_compiled(Tn=T, unroll=UNROLL):
    key = (Tn, unroll)
    if key not in _CACHE:
        _CACHE[key] = build_nc(Tn, unroll)
    return _CACHE[key]


_RUNNER_CACHE = {}


def _get_runner(nc, n_cores=NCORES):
    """Persistent jitted SPMD runner (mirrors bass2jax.run_bass_via_pjrt but
    caches the jitted callable so repeat calls skip retracing)."""
    key = id(nc)
    if key in _RUNNER_CACHE:
        return _RUNNER_CACHE[key]
    import jax
    import concourse.mybir as mybir
    from concourse import bass2jax

    bass2jax.install_neuronx_cc_hook()
    partition_name = nc.partition_id_tensor.name if nc.partition_id_tensor else None
    in_names, out_names, out_avals, zero_outs = [], [], [], []
    for alloc in nc.m.functions[0].allocations:
        if not isinstance(alloc, mybir.MemoryLocationSet):
            continue
        name = alloc.memorylocations[0].name
        if alloc.kind == "ExternalInput":
            if name != partition_name:
                in_names.append(name)
        elif alloc.kind == "ExternalOutput":
            shape = tuple(alloc.tensor_shape)
            dtype = mybir.dt.np(alloc.dtype)
            out_names.append(name)
            out_avals.append(jax.core.ShapedArray(shape, dtype))
            zero_outs.append(np.zeros(shape, dtype))
    n_params = len(in_names)
    n_outs = len(out_avals)
    all_in_names = list(in_names) + list(out_names)
    if partition_name is not None:
        all_in_names.append(partition_name)
    donate = tuple(range(n_params, n_params + n_outs))

    def _body(*args):
        operands = list(args)
        if partition_name is not None:
            operands.append(bass2jax.partition_id_tensor())
        outs = bass2jax._bass_exec_p.bind(
            *operands,
            out_avals=tuple(out_avals),
            in_names=tuple(all_in_names),
            out_names=tuple(out_names),
            lowering_input_output_aliases=(),
            sim_require_finite=True,
            sim_require_nnan=True,
            nc=nc,
        )
        return tuple(outs)

    devices = jax.devices()[:n_cores]
    mesh = bass2jax.Mesh(np.asarray(devices), ("core",))
    in_specs = (bass2jax.PartitionSpec("core"),) * (n_params + n_outs)
    out_specs = (bass2jax.PartitionSpec("core"),) * n_outs
    sharded = jax.jit(
        bass2jax.shard_map(_body, mesh=mesh, in_specs=in_specs,
                           out_specs=out_specs, check_rep=False),
        donate_argnums=donate, keep_unused=True)

    def run(in_maps):
        concat_in = [
            np.concatenate([np.asarray(in_maps[c][n]) for c in range(n_cores)], axis=0)
            for n in in_names
        ]
        concat_zeros = [
            np.zeros((n_cores * z.shape[0], *z.shape[1:]), z.dtype) for z in zero_outs
        ]
        out_arrs = sharded(*concat_in, *concat_zeros)
        out_arrs = [np.asarray(a) for a in out_arrs]
        return [
            {n: out_arrs[i].reshape(n_cores, *out_avals[i].shape)[c]
             for i, n in enumerate(out_names)}
            for c in range(n_cores)
        ]

    _RUNNER_CACHE[key] = run
    return run


def make_in_maps(x, hidden, W_in, b_in, W_hh, b_hh, Tn=T):
    x = np.asarray(x, dtype=np.float32)
    hidden = np.asarray(hidden, dtype=np.float32)
    bias = (np.asarray(b_in, dtype=np.float32) + np.asarray(b_hh, dtype=np.float32))
    W_in = np.ascontiguousarray(np.asarray(W_in, dtype=np.float32))
    W_hh = np.ascontiguousarray(np.asarray(W_hh, dtype=np.float32))
    in_maps = []
    for c in range(NCORES):
        xs = np.ascontiguousarray(
            x[:, c * BC:(c + 1) * BC, :]).reshape(Tn * BC, I)
        hs = np.ascontiguousarray(hidden[c * BC:(c + 1) * BC])
        in_maps.append({"x": xs, "hidden": hs, "w_in": W_in, "w_hh": W_hh,
                        "bias": bias})
    return in_maps


def unshard(results, Tn=T):
    outs = []
    for c in range(NCORES):
        g = results[c]["outg"]                      # (H, Tn*BC)
        outs.append(g.reshape(H, Tn, BC).transpose(1, 2, 0))
    og = np.concatenate(outs, axis=1).astype(np.float32) * np.float32(ALPHA)
    return og, og[-1]


def kernel(x, hidden, W_in, b_in, W_hh, b_hh):
    nc = _get_compiled()
    run = _get_runner(nc)
    in_maps = make_in_maps(x, hidden, W_in, b_in, W_hh, b_hh)
    results = run(in_maps)
    return unshard(results)
